# revision 29
# baseline (speedup 1.0000x reference)
"""Ternary (BitwiseLinear) matmul kernel for Trainium2, 8-core data-parallel.

y = ternary(x) @ ternary(w).T  with threshold 0.05, int-exact accumulation.

Sharding: x is split along the token dim across 8 cores (4096 tokens each);
the weight is replicated. Each core computes its y shard independently
(no collectives) and shards are concatenated on the host.

Per-core pipeline (v3):
  1. x streamed in 2MB batches (4 token tiles) on the SP HWDGE ring; w
     (4x1MB) and y stores (1MB, fp16) ride the ACT HWDGE ring.
  2. quantize on DVE (3 ops batched over the 4-tile group) -> bf16 {-1,0,1}.
  3. PE-transpose 128x128 blocks into bf16 PSUM; ACT evicts with an fp8e4
     cast -> k-major xqT tiles parked in SBUF.
  4. fp8 DoubleRow matmuls (K=256/instr, N=512) accumulate y tiles in PSUM;
     ACT evicts to fp16 (exact: |y| <= 1024 < 2048) -> batched stores.
  5. Matmuls run one 4-tile group behind transposes so the weight phase
     (same quant/transpose path, program-ordered first) never blocks the
     PE FIFO; y is returned fp16 and widened to f32 on the host.
"""

import threading

import numpy as np

N_CORES = 8
TOKENS = 32768
TOK_PER_CORE = TOKENS // N_CORES
K = 1024
O = 1024
P = 128
THR = 0.05
XB = 4                    # token tiles per DMA batch / group

_cache = {}
_lock = threading.Lock()


def _split_multi_waits(nc):
    """walrus in this env can't encode >1 sync wait on one instruction: hoist
    extra waits into single-wait NOPs on the same engine, just before the
    instruction (identical per-engine wait semantics)."""
    import concourse.mybir as mybir

    uid = 0
    for f in nc.m.functions:
        for b in f.blocks:
            out = []
            changed = False
            for inst in b.instructions:
                si = inst.sync_info
                if si is not None and si.on_wait and len(si.on_wait) > 1:
                    waits = list(si.on_wait)
                    for w in waits[:-1]:
                        uid += 1
                        out.append(mybir.InstNoOp(
                            name=f"I-waitsplit-{uid}",
                            engine=inst.engine,
                            sync_info=mybir.SyncInfo(on_wait=[w], on_update=[]),
                        ))
                    inst.sync_info = mybir.SyncInfo(
                        on_wait=[waits[-1]], on_update=list(si.on_update))
                    changed = True
                out.append(inst)
            if changed:
                b.instructions = out


def build_nc(tokens=TOK_PER_CORE, loop_n=1, skip_transpose=False,
             skip_mm=False, skip_quant=False):
    import concourse.bass as bass
    import concourse.mybir as mybir
    from concourse.masks import make_identity
    from concourse.tile import TileContext

    F32 = mybir.dt.float32
    F16 = mybir.dt.float16
    BF16 = mybir.dt.bfloat16
    FP8 = mybir.dt.float8e4
    U32 = mybir.dt.uint32
    A = mybir.AluOpType

    KB = K // P                      # 8 k-blocks of 128
    n_ttiles = tokens // P
    n_groups = n_ttiles // XB

    nc = bass.Bass()
    x = nc.dram_tensor("x", [tokens, K], F32, kind="ExternalInput")
    w = nc.dram_tensor("weight", [O, K], F32, kind="ExternalInput")
    y = nc.dram_tensor("out", [tokens, O], F16, kind="ExternalOutput")

    x2 = x.rearrange("(a p) k -> a p k", p=P)
    w2 = w.rearrange("(a p) k -> a p k", p=P)
    y2 = y.rearrange("(a p) k -> a p k", p=P)

    with TileContext(nc) as tc:
        with (
            tc.tile_pool(name="const", bufs=1) as const_pool,
            tc.tile_pool(name="wqt", bufs=1) as wqt_pool,
            tc.tile_pool(name="win", bufs=4) as win_pool,
            tc.tile_pool(name="xin", bufs=4) as xin_pool,
            tc.tile_pool(name="quant", bufs=2) as q_pool,
            tc.tile_pool(name="wquant", bufs=1) as wq_pool,
            tc.tile_pool(name="xqt", bufs=4 * XB) as xqt_pool,
            tc.tile_pool(name="yout", bufs=2) as y_pool,
            tc.tile_pool(name="psum_t", bufs=2, space="PSUM") as psumt_pool,
            tc.tile_pool(name="psum_y", bufs=3, space="PSUM") as psumy_pool,
        ):
            identity = const_pool.tile([P, P], FP8)
            make_identity(nc, identity)

            def quantize(src, nb, tag, pool=None):
                """f32 [128, nb, K] view -> ternary fp8e4 [128, nb, K]:
                v = (x <= -T) on GPSIMD (bf16), then the fused DVE STT
                computes q = (x >= T) - v in one pass."""
                pool = pool or q_pool
                q = pool.tile([P, nb, K], FP8, tag=f"{tag}_q")
                if skip_quant:
                    return q
                v = pool.tile([P, nb, K], BF16, tag=f"{tag}_v")
                nc.gpsimd.tensor_scalar(
                    out=v[:], in0=src, scalar1=-THR, scalar2=None, op0=A.is_le)
                nc.vector.scalar_tensor_tensor(
                    out=q[:], in0=src, scalar=THR, in1=v[:],
                    op0=A.is_ge, op1=A.subtract)
                return q

            def transpose_to(q, j, dst_u32):
                """q fp8 [128, nb, K] slice j -> transposed k-major fp8 with
                element step 2 (the TRN2 fp8-transpose output layout), then
                a uint32-bitcast byte copy on DVE evicts PSUM->SBUF (4 fp8
                per element, no float canonicalization of junk odd bytes)."""
                ps = psumt_pool.tile([P, KB * P * 2], FP8, tag="psT")
                if not skip_transpose:
                    for kb in range(KB):
                        out_v = ps[:, kb * 2 * P:(kb + 1) * 2 * P].rearrange(
                            "p (b c) -> p b c", c=2)[:, :, 0]
                        nc.tensor.transpose(
                            out_v, q[:, j, kb * P:(kb + 1) * P], identity)
                src = ps[:].bitcast(U32)
                src = src.rearrange("p (a b) -> p a b", a=KB)
                nc.vector.tensor_copy(dst_u32, src)

            # --- weight phase: wqT fp8 step-2 [k_part, (k_blk o 2)] ---
            wqT = wqt_pool.tile([P, KB * O * 2], FP8)
            wts = []
            for pair in range(O // (2 * P)):
                wt = win_pool.tile([P, 2, K], F32, tag="w_in")
                nc.scalar.dma_start(
                    wt[:], w2[2 * pair:2 * pair + 2].rearrange("a p k -> p a k"))
                wts.append(wt)

            def wproc(pair):
                qw = quantize(wts[pair][:], 2, "w", pool=wq_pool)
                for j in range(2):
                    ob = 2 * pair + j
                    dst = wqT[:].rearrange("p (a rest) -> p a rest", a=KB)[
                        :, :, ob * 2 * P:(ob + 1) * 2 * P].bitcast(U32)
                    transpose_to(qw, j, dst)

            # --- token loop: groups of XB=4 tiles -------------------------
            def load(g):
                xt = xin_pool.tile([P, XB, K], F32, tag="x_in")
                nc.sync.dma_start(
                    xt[:],
                    x2[XB * g:XB * (g + 1)].rearrange("a p k -> p a k"))
                return xt

            def proc(g, xt):
                qx = quantize(xt[:], XB, "x")
                outs = []
                for j in range(XB):
                    xqT = xqt_pool.tile([P, KB * P * 2], FP8, tag="xqT")
                    dst = xqT[:].bitcast(U32).rearrange(
                        "p (a b) -> p a b", a=KB)
                    transpose_to(qx, j, dst)
                    outs.append(xqT)
                return outs

            def mm(g, xqTs):
                ysb = y_pool.tile([P, XB, O], F16, tag="ysb")
                for j in range(XB):
                    xqT = xqTs[j]
                    yp = psumy_pool.tile([P, 2, 512], F32, tag="yp")
                    for oh in ([] if skip_mm else range(2)):
                        for s in range(KB // 2):
                            lhsT = xqT[:, s * 4 * P:(s + 1) * 4 * P].rearrange(
                                "p (a b c) -> p a b c", a=2, c=2)[:, :, :, 0]
                            rhs = wqT[:, (2 * s) * 2 * O:(2 * s + 2) * 2 * O
                                      ].rearrange("p (a b c) -> p a b c",
                                                  a=2, c=2)[
                                :, :, oh * 512:(oh + 1) * 512, 0]
                            nc.tensor.matmul(
                                yp[:, oh, :], lhsT, rhs,
                                start=(s == 0),
                                stop=(s == KB // 2 - 1),
                                perf_mode=mybir.MatmulPerfMode.DoubleRow,
                            )
                    nc.scalar.copy(ysb[:, j, :], yp[:])
                nc.scalar.dma_start(
                    y2[XB * g:XB * (g + 1)].rearrange("a p k -> p a k"),
                    ysb[:])

            def main_body(interleave_w):
                # Weight pairs 0-1 quantize on DVE, 2-3 on ACT: both proceed
                # concurrently while the first x groups load.  Matmuls for
                # group g are emitted after proc(g+1) (lag-1) so the PE FIFO
                # never stalls waiting on wqT or quantize.
                xts = {0: load(0), 1: load(1), 2: load(2)}
                if interleave_w:
                    for pair in range(4):
                        wproc(pair)
                xq = {0: proc(0, xts.pop(0)), 1: proc(1, xts.pop(1))}
                mm(0, xq.pop(0))
                for g in range(1, n_groups):
                    if g + 2 < n_groups:
                        xts[g + 2] = load(g + 2)
                    if g + 1 < n_groups:
                        xq[g + 1] = proc(g + 1, xts.pop(g + 1))
                    mm(g, xq.pop(g))

            # loop_n > 1 wraps the token loop in a hardware loop purely for
            # benchmarking (amortizes per-call host/PJRT overhead).
            if loop_n > 1:
                for pair in range(O // (2 * P)):
                    wproc(pair)
                with tc.For_i(0, loop_n, 1):
                    main_body(interleave_w=False)
            else:
                main_body(interleave_w=True)

    _split_multi_waits(nc)
    return nc


def _get_nc(tokens=TOK_PER_CORE):
    with _lock:
        if tokens not in _cache:
            _cache[tokens] = build_nc(tokens)
        return _cache[tokens]


def kernel(x: np.ndarray, weight: np.ndarray):
    from concourse.bass_utils import run_bass_kernel_spmd

    x = np.ascontiguousarray(x, dtype=np.float32)
    weight = np.ascontiguousarray(weight, dtype=np.float32)
    assert x.shape == (TOKENS, K) and weight.shape == (O, K)

    nc = _get_nc()
    in_maps = [
        {"x": x[i * TOK_PER_CORE:(i + 1) * TOK_PER_CORE], "weight": weight}
        for i in range(N_CORES)
    ]
    res = run_bass_kernel_spmd(nc, in_maps, core_ids=list(range(N_CORES)))
    out = np.concatenate([r["out"] for r in res.results], axis=0)
    return out.astype(np.float32)


# revision 39
# speedup vs baseline: 4.8316x; 4.8316x over previous
"""Ternary (BitwiseLinear) matmul kernel for Trainium2, 8-core data-parallel.

y = ternary(x) @ ternary(w).T  with threshold 0.05, int-exact accumulation.

Sharding: x is split along the token dim across 8 cores (4096 tokens each);
the weight is replicated. Each core computes its y shard independently
(no collectives) and shards are concatenated on the host.

Per-core pipeline (v6):
  1. x streamed in 2MB batches (4 token tiles = 1 group) on the SP HWDGE
     ring; w (4x1MB) and y stores (1MB, fp16) ride the ACT HWDGE ring.
  2. quantize to ternary fp8e4 on DVE in 1.5 passes: v = (x <= -T)
     (single-stream op, 2x two-port mode), then a fused
     scalar_tensor_tensor computes q = (x >= T) - v in one pass.
  3. PE fp8 transposes of 128x128 blocks (TRN2 writes fp8 transposes with
     element step 2) into fp8 PSUM; a uint32-bitcast byte copy on DVE
     evicts PSUM->SBUF (4 fp8/element; integer path, so the junk odd
     bytes cannot be NaN-canonicalized or denormal-flushed).  xqT keeps
     the step-2 layout; matmul APs skip the junk bytes.
  4. fp8 DoubleRow matmuls (K=256/instr, N=512, k-step-major so adjacent
     matmuls share the stationary operand; a post-pass dedupes the
     redundant LDWEIGHTS) accumulate y tiles in f32 PSUM; ACT evicts to
     fp16 (exact: |y| <= 1024 < 2048) -> 1MB batched stores.
  5. Stage lags decouple the engines: quantize(g+2) is emitted before the
     transpose/evictions of g+1 and matmuls of g each loop step, weight
     quants interleave with the first x quants in DMA-arrival order, and
     weight transposes trail (matmuls lag 2 groups, so wqT is ready).
  6. y is returned fp16 and widened to f32 on the host (exact).

Engines avoided on purpose: GPSIMD elementwise is ~10x slower than the
cost model on HW (a [128,4096] tensor_scalar measured ~50us) and GPSIMD
cannot access PSUM; ACT copies mangle uint32 bitcasts (float datapath).
"""

import threading

import numpy as np

N_CORES = 8
TOKENS = 32768
TOK_PER_CORE = TOKENS // N_CORES
K = 1024
O = 1024
P = 128
THR = 0.05
XB = 4                    # token tiles per DMA batch / group

_cache = {}
_lock = threading.Lock()


def _split_multi_waits(nc):
    """walrus in this env can't encode >1 sync wait on one instruction: hoist
    extra waits into single-wait NOPs on the same engine, just before the
    instruction (identical per-engine wait semantics)."""
    import concourse.mybir as mybir

    uid = 0
    for f in nc.m.functions:
        for b in f.blocks:
            out = []
            changed = False
            for inst in b.instructions:
                si = inst.sync_info
                if si is not None and si.on_wait and len(si.on_wait) > 1:
                    waits = list(si.on_wait)
                    for w in waits[:-1]:
                        uid += 1
                        out.append(mybir.InstNoOp(
                            name=f"I-waitsplit-{uid}",
                            engine=inst.engine,
                            sync_info=mybir.SyncInfo(on_wait=[w], on_update=[]),
                        ))
                    inst.sync_info = mybir.SyncInfo(
                        on_wait=[waits[-1]], on_update=list(si.on_update))
                    changed = True
                out.append(inst)
            if changed:
                b.instructions = out


def _dedupe_ldweights(nc):
    """Legalization emits an InstLdweights before every InstMatmult.  When
    consecutive PE matmuls use the same stationary operand, the repeated
    load is redundant -- the array keeps its weights until the next load.
    Drop LDWs identical to the previous one (PE-stream-wise), folding any
    sync waits into the following instruction (multi-waits are split by
    _split_multi_waits afterwards)."""
    import concourse.mybir as mybir

    def sig(inst):
        ap = inst.ins[0]
        return (getattr(ap, "memref", None), getattr(ap, "offset", None),
                str(getattr(ap, "ap", None)), str(getattr(ap, "dtype", None)),
                str(inst.perf_mode), str(inst.is_transpose),
                str(getattr(inst, "tile_position", None)))

    removed = 0
    for f in nc.m.functions:
        for b in f.blocks:
            out = []
            last_sig = None
            pending_waits = []
            for inst in b.instructions:
                if inst.engine != mybir.EngineType.PE:
                    out.append(inst)
                    continue
                nm = type(inst).__name__
                if nm == "InstLdweights":
                    si = inst.sync_info
                    has_upd = bool(si and si.on_update)
                    if sig(inst) == last_sig and not has_upd:
                        if si and si.on_wait:
                            pending_waits.extend(si.on_wait)
                        removed += 1
                        continue
                    last_sig = sig(inst)
                elif nm != "InstMatmult":
                    last_sig = None
                if pending_waits:
                    si = inst.sync_info
                    waits = list(si.on_wait) if si and si.on_wait else []
                    upds = list(si.on_update) if si and si.on_update else []
                    inst.sync_info = mybir.SyncInfo(
                        on_wait=pending_waits + waits, on_update=upds)
                    pending_waits = []
                out.append(inst)
            assert not pending_waits
            b.instructions = out
    return removed


def build_nc(tokens=TOK_PER_CORE, loop_n=1, skip_transpose=False,
             skip_mm=False, skip_quant=False, vq_gpsimd=False, mm_ks=None,
             alias_te=False, loop_incl_w=False):
    import concourse.bass as bass
    import concourse.mybir as mybir
    from concourse.masks import make_identity
    from concourse.tile import TileContext

    F32 = mybir.dt.float32
    F16 = mybir.dt.float16
    BF16 = mybir.dt.bfloat16
    FP8 = mybir.dt.float8e4
    U32 = mybir.dt.uint32
    A = mybir.AluOpType

    KB = K // P                      # 8 k-blocks of 128
    n_ttiles = tokens // P
    n_groups = n_ttiles // XB

    nc = bass.Bass()
    x = nc.dram_tensor("x", [tokens, K], F32, kind="ExternalInput")
    w = nc.dram_tensor("weight", [O, K], F32, kind="ExternalInput")
    y = nc.dram_tensor("out", [tokens, O], F16, kind="ExternalOutput")

    x2 = x.rearrange("(a p) k -> a p k", p=P)
    w2 = w.rearrange("(a p) k -> a p k", p=P)
    y2 = y.rearrange("(a p) k -> a p k", p=P)

    with TileContext(nc) as tc:
        with (
            tc.tile_pool(name="const", bufs=1) as const_pool,
            tc.tile_pool(name="wqt", bufs=1) as wqt_pool,
            tc.tile_pool(name="win", bufs=4) as win_pool,
            tc.tile_pool(name="xin", bufs=4) as xin_pool,
            tc.tile_pool(name="quant", bufs=2) as q_pool,
            tc.tile_pool(name="wquant", bufs=1) as wq_pool,
            tc.tile_pool(name="xqt", bufs=3 * XB + 2) as xqt_pool,
            tc.tile_pool(name="yout", bufs=2) as y_pool,
            tc.tile_pool(name="psum_t", bufs=2, space="PSUM") as psumt_pool,
            tc.tile_pool(name="psum_y", bufs=3, space="PSUM") as psumy_pool,
        ):
            identity = const_pool.tile([P, P], FP8)
            make_identity(nc, identity)

            def quantize(src, nb, tag, pool=None):
                """f32 [128, nb, K] view -> ternary fp8e4 [128, nb, K]:
                v = (x <= -T) on GPSIMD (bf16), then the fused DVE STT
                computes q = (x >= T) - v in one pass."""
                pool = pool or q_pool
                q = pool.tile([P, nb, K], FP8, tag=f"{tag}_q")
                if skip_quant:
                    return q
                v = pool.tile([P, nb, K], BF16, tag=f"{tag}_v")
                veng = nc.gpsimd if vq_gpsimd else nc.vector
                veng.tensor_scalar(
                    out=v[:], in0=src, scalar1=-THR, scalar2=None, op0=A.is_le)
                nc.vector.scalar_tensor_tensor(
                    out=q[:], in0=src, scalar=THR, in1=v[:],
                    op0=A.is_ge, op1=A.subtract)
                return q

            def transpose_to(q, j, dst_u32):
                """q fp8 [128, nb, K] slice j -> transposed k-major fp8 with
                element step 2 (the TRN2 fp8-transpose output layout), then
                a uint32-bitcast byte copy on DVE evicts PSUM->SBUF (4 fp8
                per element, no float canonicalization of junk odd bytes)."""
                ps = psumt_pool.tile([P, KB * P * 2], FP8, tag="psT")
                if not skip_transpose:
                    for kb in range(KB):
                        out_v = ps[:, kb * 2 * P:(kb + 1) * 2 * P].rearrange(
                            "p (b c) -> p b c", c=2)[:, :, 0]
                        nc.tensor.transpose(
                            out_v, q[:, j, kb * P:(kb + 1) * P], identity)
                src = ps[:].bitcast(U32)
                src = src.rearrange("p (a b) -> p a b", a=KB)
                nc.vector.tensor_copy(dst_u32, src)

            # --- weight phase: wqT fp8 step-2 [k_part, (k_blk o 2)] ---
            wqT = wqt_pool.tile([P, KB * O * 2], FP8)
            wts = []
            for pair in range(O // (2 * P)):
                wt = win_pool.tile([P, 2, K], F32, tag="w_in")
                nc.scalar.dma_start(
                    wt[:], w2[2 * pair:2 * pair + 2].rearrange("a p k -> p a k"))
                wts.append(wt)

            def wquant(pair):
                return quantize(wts[pair][:], 2, f"w{pair}", pool=wq_pool)

            def wtrans(pair, qw):
                for j in range(2):
                    ob = 2 * pair + j
                    dst = wqT[:].rearrange("p (a rest) -> p a rest", a=KB)[
                        :, :, ob * 2 * P:(ob + 1) * 2 * P].bitcast(U32)
                    transpose_to(qw, j, dst)

            def wproc(pair):
                wtrans(pair, wquant(pair))

            # --- token loop: groups of XB=4 tiles -------------------------
            def load(g):
                xt = xin_pool.tile([P, XB, K], F32, tag="x_in")
                nc.sync.dma_start(
                    xt[:],
                    x2[XB * g:XB * (g + 1)].rearrange("a p k -> p a k"))
                return xt

            def trans_evict(g, qx):
                outs = []
                for j in range(2 if alias_te else XB):
                    xqT = xqt_pool.tile([P, KB * P * 2], FP8, tag="xqT")
                    dst = xqT[:].bitcast(U32).rearrange(
                        "p (a b) -> p a b", a=KB)
                    transpose_to(qx, j, dst)
                    outs.append(xqT)
                if alias_te:
                    outs = outs + outs
                return outs

            def mm(g, xqTs):
                ysb = y_pool.tile([P, XB, O], F16, tag="ysb")
                for j in range(XB):
                    xqT = xqTs[j]
                    yp = psumy_pool.tile([P, 2, 512], F32, tag="yp")
                    nks = KB // 2 if mm_ks is None else mm_ks
                    for s in ([] if skip_mm else range(nks)):
                        for oh in range(2):
                            lhsT = xqT[:, s * 4 * P:(s + 1) * 4 * P].rearrange(
                                "p (a b c) -> p a b c", a=2, c=2)[:, :, :, 0]
                            rhs = wqT[:, (2 * s) * 2 * O:(2 * s + 2) * 2 * O
                                      ].rearrange("p (a b c) -> p a b c",
                                                  a=2, c=2)[
                                :, :, oh * 512:(oh + 1) * 512, 0]
                            nc.tensor.matmul(
                                yp[:, oh, :], lhsT, rhs,
                                start=(s == 0),
                                stop=(s == nks - 1),
                                perf_mode=mybir.MatmulPerfMode.DoubleRow,
                            )
                    nc.scalar.copy(ysb[:, j, :], yp[:])
                nc.scalar.dma_start(
                    y2[XB * g:XB * (g + 1)].rearrange("a p k -> p a k"),
                    ysb[:])

            def main_body(interleave_w):
                # Stage lags: quantize(g+2) is emitted BEFORE the transpose
                # evictions of g+1 each iteration, so the DVE queue never
                # stalls waiting on PE transposes; matmuls trail at lag 2.
                # Weight quants interleave with the first x quants in DMA
                # arrival order; weight transposes/evictions trail so the
                # (lag-2) matmuls still find wqT ready.
                xts = {0: load(0), 1: load(1), 2: load(2)}
                qs, wqs = {}, {}
                if interleave_w:
                    wqs[0] = wquant(0)
                    wqs[1] = wquant(1)
                qs[0] = quantize(xts.pop(0)[:], XB, "x")
                if interleave_w:
                    wqs[2] = wquant(2)
                    wqs[3] = wquant(3)
                qs[1] = quantize(xts.pop(1)[:], XB, "x")
                if interleave_w:
                    for pair in range(4):
                        wtrans(pair, wqs.pop(pair))
                te = {0: trans_evict(0, qs.pop(0))}
                for g in range(n_groups):
                    mm(g, te.pop(g))
                    if g + 3 < n_groups:
                        xts[g + 3] = load(g + 3)
                    if g + 2 < n_groups:
                        qs[g + 2] = quantize(xts.pop(g + 2)[:], XB, "x")
                    if g + 1 < n_groups:
                        te[g + 1] = trans_evict(g + 1, qs.pop(g + 1))

            # loop_n > 1 wraps the body in a hardware loop purely for
            # benchmarking (amortizes per-call host/PJRT overhead).
            # loop_incl_w additionally re-runs the weight phase every
            # iteration, approximating the one-shot (harness) cost.
            if loop_n > 1 and loop_incl_w:
                first = [True]

                def full_body():
                    if not first[0]:
                        for pair in range(O // (2 * P)):
                            wt = win_pool.tile([P, 2, K], F32, tag="w_in")
                            nc.scalar.dma_start(
                                wt[:],
                                w2[2 * pair:2 * pair + 2].rearrange(
                                    "a p k -> p a k"))
                            wts[pair] = wt
                    first[0] = False
                    main_body(interleave_w=True)

                with tc.For_i(0, loop_n, 1):
                    full_body()
            elif loop_n > 1:
                for pair in range(O // (2 * P)):
                    wproc(pair)
                with tc.For_i(0, loop_n, 1):
                    main_body(interleave_w=False)
            else:
                main_body(interleave_w=True)

    _dedupe_ldweights(nc)
    _split_multi_waits(nc)
    return nc


def _get_nc(tokens=TOK_PER_CORE):
    with _lock:
        if tokens not in _cache:
            _cache[tokens] = build_nc(tokens)
        return _cache[tokens]


def kernel(x: np.ndarray, weight: np.ndarray):
    from concourse.bass_utils import run_bass_kernel_spmd

    x = np.ascontiguousarray(x, dtype=np.float32)
    weight = np.ascontiguousarray(weight, dtype=np.float32)
    assert x.shape == (TOKENS, K) and weight.shape == (O, K)

    nc = _get_nc()
    in_maps = [
        {"x": x[i * TOK_PER_CORE:(i + 1) * TOK_PER_CORE], "weight": weight}
        for i in range(N_CORES)
    ]
    res = run_bass_kernel_spmd(nc, in_maps, core_ids=list(range(N_CORES)))
    out = np.concatenate([r["out"] for r in res.results], axis=0)
    return out.astype(np.float32)
